# revision 39
# baseline (speedup 1.0000x reference)
"""Trainium2 Bass kernel for GQA attention with RoPE (dense_transformer).

Problem: bs=2, seq=2048, dim=2048, 16 Q heads / 8 KV heads, head_dim=128,
causal, scores scaled by 1/sqrt(dim), f32 I/O.

Sharding: 8 cores = 2 (batch) x 4 (head groups). Each core computes 4 Q heads
+ its 2 matching KV heads for one batch element. No collectives.

Per-core design (all matmul inputs bf16, PSUM accumulation f32):
- Host supplies hidden_state TRANSPOSED (hsT: [dim, seq]) so the contraction
  dim is always on partitions; the kernel never transposes anything.
- wq/wk columns are host-permuted per head so each head's rows are
  [re0..15, im0..15] interleaved per 32-row quadrant; RoPE's (even,odd)
  pairing then becomes a 16-row swap within each quadrant, done with a single
  DVE stream_shuffle: rot = A*CC + shuffle16(A*SS).
- Scores are computed transposed (scoresT[s2, s1] = kT_tile.T @ qT) so softmax
  needs only a cross-partition SUM, obtained for free by appending a ones
  column to V (denominator accumulates in the same PV matmul).
- exp has no max-subtraction (scores are ~N(0, 1/16); safe in f32/bf16).
- Causal masking: above-diagonal 128-col blocks are never computed; diagonal
  128x128 squares get one bf16 mask multiply.
"""

import math
import os
import sys

for _p in ("/root/.axon_site", "/root/.axon_site/_ro/trn_rl_repo",
           "/root/.axon_site/_ro/pypackages"):
    if os.path.isdir(_p) and _p not in sys.path:
        sys.path.append(_p)

# The kernel executes through jax's axon backend; if a harness pinned
# JAX_PLATFORMS=cpu (the single-core bench convention), restore platform
# auto-detection before jax initializes.
if "jax" not in sys.modules and "axon" not in os.environ.get("JAX_PLATFORMS", "axon"):
    os.environ.pop("JAX_PLATFORMS", None)

import numpy as np
import ml_dtypes

from concourse import bass, bacc, mybir, tile
from concourse.bass_utils import run_bass_kernel_spmd

BF16 = ml_dtypes.bfloat16
F32 = np.float32

N_HEADS, N_KV, HD = 16, 8, 128
BS, S, D = 2, 2048, 2048
ROPE_BASE = 10000.0
P = 128
KT = D // P          # 16 contraction k-tiles
NQ = N_HEADS // 4    # 4 q heads per core
NKV = N_KV // 4      # 2 kv heads per core
CH = S // 512        # 4 query chunks of 512
SCALE = 1.0 / math.sqrt(D)

_SHUF16 = list(range(16, 32)) + list(range(16))


def _perm():
    p = np.arange(P)
    quad, j = p // 32, p % 32
    is_im = (j % 32) >= 16
    fidx = 16 * quad + (j % 16)
    orig = np.where(is_im, 2 * fidx + 1, 2 * fidx)
    return orig, fidx, is_im


_ORIG, _FIDX, _IS_IM = _perm()


def _cc_ss():
    inv_freq = ROPE_BASE ** (-np.arange(0, HD, 2, dtype=np.float64) / HD)
    pos = np.arange(S, dtype=np.float64)
    ang = pos[None, :] * inv_freq[_FIDX][:, None]          # [128, S]
    cc = np.cos(ang).astype(F32)
    ss = (np.sin(ang) * np.where(_IS_IM, -1.0, 1.0)[:, None]).astype(F32)
    return np.ascontiguousarray(cc), np.ascontiguousarray(ss)


_CC, _SS = _cc_ss()
_MASK = np.ascontiguousarray(
    (np.arange(128)[None, :] >= np.arange(128)[:, None]).astype(BF16))  # f >= p


def _build_nc(with_bias: bool, repeat: int = 1):
    nc = bacc.Bacc("TRN2", target_bir_lowering=False, debug=False, num_devices=8)
    bf = mybir.dt.bfloat16
    f32 = mybir.dt.float32

    hst_d = nc.declare_dram_parameter("hst", [P, KT, S], bf, isOutput=False)
    wq_d = nc.declare_dram_parameter("wq", [P, KT, NQ * HD], bf, isOutput=False)
    wk_d = nc.declare_dram_parameter("wk", [P, KT, NKV * HD], bf, isOutput=False)
    wv_d = nc.declare_dram_parameter("wv", [P, KT, NKV * HD], bf, isOutput=False)
    cc_d = nc.declare_dram_parameter("cc", [P, S], f32, isOutput=False)
    ss_d = nc.declare_dram_parameter("ss", [P, S], f32, isOutput=False)
    mask_d = nc.declare_dram_parameter("mask", [P, P], bf, isOutput=False)
    if with_bias:
        bq_d = nc.declare_dram_parameter("bq", [P, NQ], f32, isOutput=False)
        bk_d = nc.declare_dram_parameter("bk", [P, NKV], f32, isOutput=False)
        bv_d = nc.declare_dram_parameter("bv", [P, NKV, HD], f32, isOutput=False)
    out_d = nc.declare_dram_parameter("out", [S, NQ * HD], f32, isOutput=True)

    with tile.TileContext(nc) as tc:
        with tc.tile_pool(name="singles", bufs=1) as singles:
            hst_sb = singles.tile([P, KT, S], bf, tag="hst")
            wq_sb = singles.tile([P, KT, NQ * HD], bf, tag="wq")
            wk_sb = singles.tile([P, KT, NKV * HD], bf, tag="wk")
            wv_sb = singles.tile([P, KT, NKV * HD], bf, tag="wv")
            cc_sb = singles.tile([P, S], f32, tag="cc")
            ss_sb = singles.tile([P, S], f32, tag="ss")
            mask_sb = singles.tile([P, P], bf, tag="mask")
            qrope = singles.tile([P, NQ, S], bf, tag="qrope")
            krope = singles.tile([P, NKV, S], bf, tag="krope")
            vaug = singles.tile([P, NKV, KT, HD + 1], bf, tag="vaug")

            nc.sync.dma_start(out=wk_sb[:, 0:KT // 2, :],
                              in_=wk_d[:, 0:KT // 2, :])
            nc.sync.dma_start(out=wk_sb[:, KT // 2:, :],
                              in_=wk_d[:, KT // 2:, :])
            for k in range(KT):
                nc.sync.dma_start(out=hst_sb[:, k, :], in_=hst_d[:, k, :])
            nc.sync.dma_start(out=cc_sb[:], in_=cc_d[:])
            nc.sync.dma_start(out=ss_sb[:], in_=ss_d[:])
            nc.sync.dma_start(out=wv_sb[:], in_=wv_d[:])
            nc.sync.dma_start(out=wq_sb[:], in_=wq_d[:])
            nc.sync.dma_start(out=mask_sb[:], in_=mask_d[:])
            if with_bias:
                bq_sb = singles.tile([P, NQ], f32, tag="bq")
                bk_sb = singles.tile([P, NKV], f32, tag="bk")
                bv_sb = singles.tile([P, NKV, HD], f32, tag="bv")
                nc.sync.dma_start(out=bq_sb[:], in_=bq_d[:])
                nc.sync.dma_start(out=bk_sb[:], in_=bk_d[:])
                nc.sync.dma_start(out=bv_sb[:], in_=bv_d[:])

            nc.vector.memset(vaug[:, :, :, HD:HD + 1], 1.0)

            for _rep in range(repeat):
                _emit_body(nc, tc, with_bias, locals())

    nc.compile()
    return nc


def _emit_body(nc, tc, with_bias, env):
    bf = mybir.dt.bfloat16
    f32 = mybir.dt.float32
    hst_sb, wq_sb, wk_sb, wv_sb = env["hst_sb"], env["wq_sb"], env["wk_sb"], env["wv_sb"]
    cc_sb, ss_sb, mask_sb = env["cc_sb"], env["ss_sb"], env["mask_sb"]
    qrope, krope, vaug, out_d = env["qrope"], env["krope"], env["vaug"], env["out_d"]
    bq_sb = env.get("bq_sb"); bk_sb = env.get("bk_sb"); bv_sb = env.get("bv_sb")
    if True:
        if True:
            # ---------- Stage 1: K projection, k-outer so each hst k-tile is
            # consumed as soon as its DMA lands, and each wk weight tile stays
            # stationary for 4 chunk matmuls.
            with tc.tile_pool(name="ropet", bufs=4) as ropet:

                def rope(ps, out_ap, c, bias_ap):
                    csl = slice(512 * c, 512 * (c + 1))
                    t1 = ropet.tile([P, 512], f32, tag="t1")
                    t2 = ropet.tile([P, 512], f32, tag="t2")
                    t2s = ropet.tile([P, 512], f32, tag="t2s")
                    if bias_ap is None:
                        nc.vector.tensor_mul(t1[:], ps[:], cc_sb[:, csl])
                        nc.vector.tensor_mul(t2[:], ps[:], ss_sb[:, csl])
                    else:
                        nc.vector.scalar_tensor_tensor(
                            t1[:], ps[:], bias_ap, cc_sb[:, csl],
                            mybir.AluOpType.add, mybir.AluOpType.mult)
                        nc.vector.scalar_tensor_tensor(
                            t2[:], ps[:], bias_ap, ss_sb[:, csl],
                            mybir.AluOpType.add, mybir.AluOpType.mult)
                    nc.vector.stream_shuffle(t2s[:], t2[:], _SHUF16)
                    nc.vector.tensor_add(out_ap, t1[:], t2s[:])

                # Single unified PSUM pool (8 banks, one tag) so slots
                # hand off tile-by-tile between projection, scores and PV
                # accumulators with per-slot WAR waits instead of pool-level
                # barriers.
                with tc.tile_pool(name="ps", bufs=8, space="PSUM") as psP, \
                     tc.tile_pool(name="probs", bufs=6) as probs_pool, \
                     tc.tile_pool(name="outp", bufs=8) as outp, \
                     tc.tile_pool(name="misc", bufs=8) as misc:
                    # K projection, k-outer: all 8 (m,c) groups accumulate at
                    # once; each hst k-tile is consumed as its DMA lands.
                    NG = NKV * CH
                    kps = [psP.tile([P, 512], f32, tag="ps",
                                    name=f"psK_{m}_{c}")
                           for m in range(NKV) for c in range(CH)
                           if m * CH + c < NG - 1]
                    for k in range(KT):
                        for g in range(NG - 1):
                            m, c = divmod(g, CH)
                            nc.tensor.matmul(
                                kps[g][:],
                                lhsT=wk_sb[:, k, HD * m:HD * (m + 1)],
                                rhs=hst_sb[:, k, 512 * c:512 * (c + 1)],
                                start=(k == 0), stop=(k == KT - 1))
                    lastg = psP.tile([P, 512], f32, tag="ps", name="psK_last")
                    m, c = divmod(NG - 1, CH)
                    for k in range(KT):
                        nc.tensor.matmul(
                            lastg[:],
                            lhsT=wk_sb[:, k, HD * m:HD * (m + 1)],
                            rhs=hst_sb[:, k, 512 * c:512 * (c + 1)],
                            start=(k == 0), stop=(k == KT - 1))
                    for g in range(NG - 1):
                        m, c = divmod(g, CH)
                        rope(kps[g], krope[:, m, 512 * c:512 * (c + 1)], c,
                             bk_sb[:, m:m + 1] if with_bias else None)
                    m, c = divmod(NG - 1, CH)
                    rope(lastg, krope[:, m, 512 * c:512 * (c + 1)], c,
                         bk_sb[:, m:m + 1] if with_bias else None)

                    # V projection (layout [s2, hd]); ones column already set
                    for j in range(KT):
                        ps = psP.tile([P, 512], f32, tag="ps")
                        for k in range(KT):
                            nc.tensor.matmul(
                                ps[:, 0:NKV * HD],
                                lhsT=hst_sb[:, k, P * j:P * (j + 1)],
                                rhs=wv_sb[:, k, :],
                                start=(k == 0), stop=(k == KT - 1))
                        for kv in range(NKV):
                            nc.scalar.copy(vaug[:, kv, j, 0:HD],
                                           ps[:, HD * kv:HD * (kv + 1)])
                        if with_bias:
                            for kv in range(NKV):
                                nc.vector.tensor_add(vaug[:, kv, j, 0:HD],
                                                     vaug[:, kv, j, 0:HD],
                                                     bv_sb[:, kv, :])

                    def emit_qproj(h, c):
                        ps = psP.tile([P, 512], f32, tag="ps",
                                      name=f"psQ_{h}_{c}")
                        for k in range(KT):
                            nc.tensor.matmul(
                                ps[:],
                                lhsT=wq_sb[:, k, HD * h:HD * (h + 1)],
                                rhs=hst_sb[:, k, 512 * c:512 * (c + 1)],
                                start=(k == 0), stop=(k == KT - 1))
                        rope(ps, qrope[:, h, 512 * c:512 * (c + 1)], c,
                             bq_sb[:, h:h + 1] if with_bias else None)

                    for c in range(CH):
                        emit_qproj(0, c)

                    def head_c_order(h):
                        # last head runs big chunk first so the kernel tail is
                        # the smallest chunk's drain
                        return (list(range(CH)) if h + 1 < NQ
                                else list(range(CH - 1, -1, -1)))

                    for h in range(NQ):
                        # attention for this head; Q projection groups of the
                        # next head interleave between attention chunks so the
                        # PE fills ACT-exp latency windows
                        kv = h // 2
                        c_order = head_c_order(h)
                        for ci, c in enumerate(c_order):
                            ops = [psP.tile([P, 512], f32, tag="ps",
                                            name=f"psO_{h}_{c}_{ir}")[:, 0:HD + 1]
                                   for ir in range(4)]
                            ot = outp.tile([P, 4, HD], f32, tag="ot",
                                           name=f"ot_{h}_{c}")

                            def drain(ir):
                                rec = misc.tile([P, 1], f32, tag="rec")
                                nc.vector.reciprocal(rec[:],
                                                     ops[ir][:, HD:HD + 1])
                                nc.vector.tensor_scalar_mul(
                                    ot[:, ir, :], ops[ir][:, 0:HD], rec[:])
                                if ir == 3:
                                    nc.sync.dma_start(
                                        out=out_d[512 * c:512 * (c + 1),
                                                  HD * h:HD * (h + 1)]
                                        .rearrange("(ir p) n -> p ir n", p=P),
                                        in_=ot[:])

                            # software-pipelined j loop: scores for j are
                            # issued before PV matmuls of j-1, so the PE never
                            # stalls on the ACT exp of the current tile
                            def emit_scores(j):
                                lo = max(j - 4 * c, 0) * P
                                sps = psP.tile([P, 512], f32, tag="ps",
                                               name=f"sps_{h}_{c}_{j}")
                                nc.tensor.matmul(
                                    sps[:, lo:512],
                                    lhsT=krope[:, kv, P * j:P * (j + 1)],
                                    rhs=qrope[:, h, 512 * c + lo:512 * (c + 1)],
                                    start=True, stop=True)
                                return sps

                            def emit_probs_pv(j, sps):
                                jj = j - 4 * c
                                lo = max(jj, 0) * P
                                pt = probs_pool.tile([P, 512], bf, tag="probs")
                                nc.scalar.activation(
                                    pt[:, lo:512], sps[:, lo:512],
                                    mybir.ActivationFunctionType.Exp,
                                    bias=0.0, scale=SCALE)

                                def pv(ir):
                                    nc.tensor.matmul(
                                        ops[ir][:],
                                        lhsT=pt[:, P * ir:P * (ir + 1)],
                                        rhs=vaug[:, kv, j, :],
                                        start=(j == 0),
                                        stop=(j == 4 * c + ir))

                                # PVs on fully-valid columns first (no mask
                                # dependency), then the masked diagonal PV;
                                # accumulator jj stops here, so drain it right
                                # away to free its PSUM slot early
                                for ir in range(4):
                                    if 4 * c + ir >= j and ir != jj:
                                        pv(ir)
                                if jj >= 0:
                                    nc.vector.tensor_mul(
                                        pt[:, lo:lo + P], pt[:, lo:lo + P],
                                        mask_sb[:])
                                    pv(jj)
                                    drain(jj)

                            nj = 4 * c + 4
                            depth = 3 if nj > 3 else (2 if nj > 2 else 1)
                            pending = []
                            for j in range(nj):
                                pending.append((j, emit_scores(j)))
                                if len(pending) > depth:
                                    emit_probs_pv(*pending.pop(0))
                            for pj, psps in pending:
                                emit_probs_pv(pj, psps)
                            if h + 1 < NQ:
                                emit_qproj(h + 1, head_c_order(h + 1)[ci])


_NC_CACHE = {}


def _get_nc(with_bias: bool, repeat: int = 1):
    key = (with_bias, repeat)
    if key not in _NC_CACHE:
        _NC_CACHE[key] = _build_nc(with_bias, repeat)
    return _NC_CACHE[key]


def _make_in_maps(hidden_state, wq, bq, wk, bk, wv, bv, with_bias):
    in_maps = []
    for core in range(8):
        b, hg = core // 4, core % 4
        hst = np.ascontiguousarray(
            hidden_state[b].T.astype(BF16).reshape(KT, P, S).transpose(1, 0, 2))
        wq_cols = np.concatenate(
            [wq[:, (4 * hg + h) * HD + _ORIG] for h in range(NQ)], axis=1)
        wk_cols = np.concatenate(
            [wk[:, (2 * hg + m) * HD + _ORIG] for m in range(NKV)], axis=1)
        wv_cols = wv[:, NKV * HD * hg: NKV * HD * (hg + 1)]

        def wlay(w):
            return np.ascontiguousarray(
                w.astype(BF16).reshape(KT, P, w.shape[1]).transpose(1, 0, 2))

        m = {
            "hst": hst,
            "wq": wlay(wq_cols),
            "wk": wlay(wk_cols),
            "wv": wlay(wv_cols),
            "cc": _CC,
            "ss": _SS,
            "mask": _MASK,
        }
        if with_bias:
            m["bq"] = np.ascontiguousarray(
                np.stack([bq[(4 * hg + h) * HD + _ORIG] for h in range(NQ)],
                         axis=1).astype(F32))
            m["bk"] = np.ascontiguousarray(
                np.stack([bk[(2 * hg + mm) * HD + _ORIG] for mm in range(NKV)],
                         axis=1).astype(F32))
            m["bv"] = np.ascontiguousarray(
                np.broadcast_to(
                    bv[NKV * HD * hg: NKV * HD * (hg + 1)].reshape(1, NKV, HD),
                    (P, NKV, HD)).astype(F32))
        in_maps.append(m)
    return in_maps


def _run(hidden_state, wq, bq, wk, bk, wv, bv, trace=False, **spmd_kwargs):
    hidden_state = np.asarray(hidden_state, dtype=F32)
    wq = np.asarray(wq, dtype=F32)
    wk = np.asarray(wk, dtype=F32)
    wv = np.asarray(wv, dtype=F32)
    bq = np.asarray(bq, dtype=F32)
    bk = np.asarray(bk, dtype=F32)
    bv = np.asarray(bv, dtype=F32)
    with_bias = bool(np.any(bq) or np.any(bk) or np.any(bv))
    nc = _get_nc(with_bias)
    in_maps = _make_in_maps(hidden_state, wq, bq, wk, bk, wv, bv, with_bias)
    res = run_bass_kernel_spmd(nc, in_maps, core_ids=list(range(8)),
                               trace=trace, **spmd_kwargs)
    out = np.zeros((BS, S, N_HEADS, HD), F32)
    for core in range(8):
        b, hg = core // 4, core % 4
        oc = np.asarray(res.results[core]["out"], dtype=F32)
        out[b, :, 4 * hg:4 * hg + 4, :] = oc.reshape(S, NQ, HD)
    return out, res


def kernel(hidden_state, wq, bq, wk, bk, wv, bv):
    out, _ = _run(hidden_state, wq, bq, wk, bk, wv, bv, trace=False)
    return out


# revision 41
# speedup vs baseline: 1.2857x; 1.2857x over previous
"""Trainium2 Bass kernel for GQA attention with RoPE (dense_transformer).

Problem: bs=2, seq=2048, dim=2048, 16 Q heads / 8 KV heads, head_dim=128,
causal, scores scaled by 1/sqrt(dim), f32 I/O.

Sharding: 8 cores = 2 (batch) x 4 (head groups). Each core computes 4 Q heads
+ its 2 matching KV heads for one batch element. No collectives.

Per-core design (all matmul inputs bf16, PSUM accumulation f32):
- Host supplies hidden_state TRANSPOSED (hsT: [dim, seq]) so the contraction
  dim is always on partitions; the kernel never transposes anything.
- wq/wk columns are host-permuted per head so each head's rows are
  [re0..15, im0..15] interleaved per 32-row quadrant; RoPE's (even,odd)
  pairing then becomes a 16-row swap within each quadrant, done with a single
  DVE stream_shuffle: rot = A*CC + shuffle16(A*SS).
- Scores are computed transposed (scoresT[s2, s1] = kT_tile.T @ qT) so softmax
  needs only a cross-partition SUM, obtained for free by appending a ones
  column to V (denominator accumulates in the same PV matmul).
- exp has no max-subtraction (scores are ~N(0, 1/16); safe in f32/bf16).
- Causal masking: above-diagonal 128-col blocks are never computed; diagonal
  128x128 squares get one bf16 mask multiply.
"""

import math
import os
import sys

for _p in ("/root/.axon_site", "/root/.axon_site/_ro/trn_rl_repo",
           "/root/.axon_site/_ro/pypackages"):
    if os.path.isdir(_p) and _p not in sys.path:
        sys.path.append(_p)

# The kernel executes through jax's axon backend; if a harness pinned
# JAX_PLATFORMS=cpu (the single-core bench convention), restore platform
# auto-detection before jax initializes.
if "jax" not in sys.modules and "axon" not in os.environ.get("JAX_PLATFORMS", "axon"):
    os.environ.pop("JAX_PLATFORMS", None)

import numpy as np
import ml_dtypes

from concourse import bass, bacc, mybir, tile
from concourse.bass_utils import run_bass_kernel_spmd

BF16 = ml_dtypes.bfloat16
F32 = np.float32

N_HEADS, N_KV, HD = 16, 8, 128
BS, S, D = 2, 2048, 2048
ROPE_BASE = 10000.0
P = 128
KT = D // P          # 16 contraction k-tiles
NQ = N_HEADS // 4    # 4 q heads per core
NKV = N_KV // 4      # 2 kv heads per core
CH = S // 512        # 4 query chunks of 512
SCALE = 1.0 / math.sqrt(D)

_SHUF16 = list(range(16, 32)) + list(range(16))


def _perm():
    p = np.arange(P)
    quad, j = p // 32, p % 32
    is_im = (j % 32) >= 16
    fidx = 16 * quad + (j % 16)
    orig = np.where(is_im, 2 * fidx + 1, 2 * fidx)
    return orig, fidx, is_im


_ORIG, _FIDX, _IS_IM = _perm()


def _cc_ss():
    inv_freq = ROPE_BASE ** (-np.arange(0, HD, 2, dtype=np.float64) / HD)
    pos = np.arange(S, dtype=np.float64)
    ang = pos[None, :] * inv_freq[_FIDX][:, None]          # [128, S]
    cc = np.cos(ang).astype(F32)
    ss = (np.sin(ang) * np.where(_IS_IM, -1.0, 1.0)[:, None]).astype(F32)
    return np.ascontiguousarray(cc), np.ascontiguousarray(ss)


_CC, _SS = _cc_ss()
_MASK = np.ascontiguousarray(
    (np.arange(128)[None, :] >= np.arange(128)[:, None]).astype(BF16))  # f >= p


def _build_nc(with_bias: bool, repeat: int = 1):
    nc = bacc.Bacc("TRN2", target_bir_lowering=False, debug=False, num_devices=8)
    bf = mybir.dt.bfloat16
    f32 = mybir.dt.float32

    hst_d = nc.declare_dram_parameter("hst", [P, KT, S], bf, isOutput=False)
    wq_d = nc.declare_dram_parameter("wq", [P, KT, NQ * HD], bf, isOutput=False)
    wk_d = nc.declare_dram_parameter("wk", [P, KT, NKV * HD], bf, isOutput=False)
    wv_d = nc.declare_dram_parameter("wv", [P, KT, NKV * HD], bf, isOutput=False)
    cc_d = nc.declare_dram_parameter("cc", [P, S], f32, isOutput=False)
    ss_d = nc.declare_dram_parameter("ss", [P, S], f32, isOutput=False)
    mask_d = nc.declare_dram_parameter("mask", [P, P], bf, isOutput=False)
    if with_bias:
        bq_d = nc.declare_dram_parameter("bq", [P, NQ], f32, isOutput=False)
        bk_d = nc.declare_dram_parameter("bk", [P, NKV], f32, isOutput=False)
        bv_d = nc.declare_dram_parameter("bv", [P, NKV, HD], f32, isOutput=False)
    out_d = nc.declare_dram_parameter("out", [S, NQ * HD], f32, isOutput=True)

    with tile.TileContext(nc) as tc:
        with tc.tile_pool(name="singles", bufs=1) as singles:
            hst_sb = singles.tile([P, KT, S], bf, tag="hst")
            wq_sb = singles.tile([P, KT, NQ * HD], bf, tag="wq")
            wk_sb = singles.tile([P, KT, NKV * HD], bf, tag="wk")
            wv_sb = singles.tile([P, KT, NKV * HD], bf, tag="wv")
            cc_sb = singles.tile([P, S], f32, tag="cc")
            ss_sb = singles.tile([P, S], f32, tag="ss")
            mask_sb = singles.tile([P, P], bf, tag="mask")
            qrope = singles.tile([P, NQ, S], bf, tag="qrope")
            krope = singles.tile([P, NKV, S], bf, tag="krope")
            vaug = singles.tile([P, NKV, KT, HD + 1], bf, tag="vaug")

            nc.sync.dma_start(out=wk_sb[:, 0:KT // 2, :],
                              in_=wk_d[:, 0:KT // 2, :])
            nc.sync.dma_start(out=wk_sb[:, KT // 2:, :],
                              in_=wk_d[:, KT // 2:, :])
            for k in range(KT):
                nc.sync.dma_start(out=hst_sb[:, k, :], in_=hst_d[:, k, :])
            nc.sync.dma_start(out=cc_sb[:], in_=cc_d[:])
            nc.sync.dma_start(out=ss_sb[:], in_=ss_d[:])
            nc.sync.dma_start(out=wv_sb[:], in_=wv_d[:])
            nc.sync.dma_start(out=wq_sb[:], in_=wq_d[:])
            nc.sync.dma_start(out=mask_sb[:], in_=mask_d[:])
            if with_bias:
                bq_sb = singles.tile([P, NQ], f32, tag="bq")
                bk_sb = singles.tile([P, NKV], f32, tag="bk")
                bv_sb = singles.tile([P, NKV, HD], f32, tag="bv")
                nc.sync.dma_start(out=bq_sb[:], in_=bq_d[:])
                nc.sync.dma_start(out=bk_sb[:], in_=bk_d[:])
                nc.sync.dma_start(out=bv_sb[:], in_=bv_d[:])

            nc.vector.memset(vaug[:, :, :, HD:HD + 1], 1.0)

            for _rep in range(repeat):
                _emit_body(nc, tc, with_bias, locals())

    nc.compile()
    return nc


def _emit_body(nc, tc, with_bias, env):
    bf = mybir.dt.bfloat16
    f32 = mybir.dt.float32
    hst_sb, wq_sb, wk_sb, wv_sb = env["hst_sb"], env["wq_sb"], env["wk_sb"], env["wv_sb"]
    cc_sb, ss_sb, mask_sb = env["cc_sb"], env["ss_sb"], env["mask_sb"]
    qrope, krope, vaug, out_d = env["qrope"], env["krope"], env["vaug"], env["out_d"]
    bq_sb = env.get("bq_sb"); bk_sb = env.get("bk_sb"); bv_sb = env.get("bv_sb")
    if True:
        if True:
            # ---------- Stage 1: K projection, k-outer so each hst k-tile is
            # consumed as soon as its DMA lands, and each wk weight tile stays
            # stationary for 4 chunk matmuls.
            with tc.tile_pool(name="ropet", bufs=4) as ropet:

                def rope(ps, out_ap, c, bias_ap):
                    csl = slice(512 * c, 512 * (c + 1))
                    t1 = ropet.tile([P, 512], f32, tag="t1")
                    t2 = ropet.tile([P, 512], f32, tag="t2")
                    t2s = ropet.tile([P, 512], f32, tag="t2s")
                    if bias_ap is None:
                        nc.vector.tensor_mul(t1[:], ps[:], cc_sb[:, csl])
                        nc.vector.tensor_mul(t2[:], ps[:], ss_sb[:, csl])
                    else:
                        nc.vector.scalar_tensor_tensor(
                            t1[:], ps[:], bias_ap, cc_sb[:, csl],
                            mybir.AluOpType.add, mybir.AluOpType.mult)
                        nc.vector.scalar_tensor_tensor(
                            t2[:], ps[:], bias_ap, ss_sb[:, csl],
                            mybir.AluOpType.add, mybir.AluOpType.mult)
                    nc.vector.stream_shuffle(t2s[:], t2[:], _SHUF16)
                    nc.vector.tensor_add(out_ap, t1[:], t2s[:])

                # Single unified PSUM pool (8 banks, one tag) so slots
                # hand off tile-by-tile between projection, scores and PV
                # accumulators with per-slot WAR waits instead of pool-level
                # barriers.
                with tc.tile_pool(name="ps", bufs=8, space="PSUM") as psP, \
                     tc.tile_pool(name="probs", bufs=6) as probs_pool, \
                     tc.tile_pool(name="outp", bufs=8) as outp, \
                     tc.tile_pool(name="misc", bufs=8) as misc:
                    # K projection, k-outer: all 8 (m,c) groups accumulate at
                    # once; each hst k-tile is consumed as its DMA lands.
                    NG = NKV * CH
                    kps = [psP.tile([P, 512], f32, tag="ps",
                                    name=f"psK_{m}_{c}")
                           for m in range(NKV) for c in range(CH)
                           if m * CH + c < NG - 1]
                    for k in range(KT):
                        for g in range(NG - 1):
                            m, c = divmod(g, CH)
                            nc.tensor.matmul(
                                kps[g][:],
                                lhsT=wk_sb[:, k, HD * m:HD * (m + 1)],
                                rhs=hst_sb[:, k, 512 * c:512 * (c + 1)],
                                start=(k == 0), stop=(k == KT - 1))
                    lastg = psP.tile([P, 512], f32, tag="ps", name="psK_last")
                    m, c = divmod(NG - 1, CH)
                    for k in range(KT):
                        nc.tensor.matmul(
                            lastg[:],
                            lhsT=wk_sb[:, k, HD * m:HD * (m + 1)],
                            rhs=hst_sb[:, k, 512 * c:512 * (c + 1)],
                            start=(k == 0), stop=(k == KT - 1))
                    for g in range(NG - 1):
                        m, c = divmod(g, CH)
                        rope(kps[g], krope[:, m, 512 * c:512 * (c + 1)], c,
                             bk_sb[:, m:m + 1] if with_bias else None)
                    m, c = divmod(NG - 1, CH)
                    rope(lastg, krope[:, m, 512 * c:512 * (c + 1)], c,
                         bk_sb[:, m:m + 1] if with_bias else None)

                    # V projection (layout [s2, hd]); ones column already set
                    for j in range(KT):
                        ps = psP.tile([P, 512], f32, tag="ps")
                        for k in range(KT):
                            nc.tensor.matmul(
                                ps[:, 0:NKV * HD],
                                lhsT=hst_sb[:, k, P * j:P * (j + 1)],
                                rhs=wv_sb[:, k, :],
                                start=(k == 0), stop=(k == KT - 1))
                        for kv in range(NKV):
                            nc.scalar.copy(vaug[:, kv, j, 0:HD],
                                           ps[:, HD * kv:HD * (kv + 1)])
                        if with_bias:
                            for kv in range(NKV):
                                nc.vector.tensor_add(vaug[:, kv, j, 0:HD],
                                                     vaug[:, kv, j, 0:HD],
                                                     bv_sb[:, kv, :])

                    def emit_qproj(h, c):
                        ps = psP.tile([P, 512], f32, tag="ps",
                                      name=f"psQ_{h}_{c}")
                        for k in range(KT):
                            nc.tensor.matmul(
                                ps[:],
                                lhsT=wq_sb[:, k, HD * h:HD * (h + 1)],
                                rhs=hst_sb[:, k, 512 * c:512 * (c + 1)],
                                start=(k == 0), stop=(k == KT - 1))
                        rope(ps, qrope[:, h, 512 * c:512 * (c + 1)], c,
                             bq_sb[:, h:h + 1] if with_bias else None)

                    for c in range(CH):
                        emit_qproj(0, c)

                    def head_c_order(h):
                        # last head runs big chunk first so the kernel tail is
                        # the smallest chunk's drain
                        return (list(range(CH)) if h + 1 < NQ
                                else list(range(CH - 1, -1, -1)))

                    for h in range(NQ):
                        # attention for this head; Q projection groups of the
                        # next head interleave between attention chunks so the
                        # PE fills ACT-exp latency windows
                        kv = h // 2
                        c_order = head_c_order(h)
                        for ci, c in enumerate(c_order):
                            ops = [psP.tile([P, 512], f32, tag="ps",
                                            name=f"psO_{h}_{c}_{ir}")[:, 0:HD + 1]
                                   for ir in range(4)]
                            ot = outp.tile([P, 4, HD], f32, tag="ot",
                                           name=f"ot_{h}_{c}")

                            def drain(ir):
                                rec = misc.tile([P, 1], f32, tag="rec")
                                nc.vector.reciprocal(rec[:],
                                                     ops[ir][:, HD:HD + 1])
                                nc.vector.tensor_scalar_mul(
                                    ot[:, ir, :], ops[ir][:, 0:HD], rec[:])
                                if ir == 3:
                                    nc.sync.dma_start(
                                        out=out_d[512 * c:512 * (c + 1),
                                                  HD * h:HD * (h + 1)]
                                        .rearrange("(ir p) n -> p ir n", p=P),
                                        in_=ot[:])

                            # software-pipelined j loop: scores for j are
                            # issued before PV matmuls of j-1, so the PE never
                            # stalls on the ACT exp of the current tile
                            def emit_scores(j):
                                lo = max(j - 4 * c, 0) * P
                                sps = psP.tile([P, 512], f32, tag="ps",
                                               name=f"sps_{h}_{c}_{j}")
                                nc.tensor.matmul(
                                    sps[:, lo:512],
                                    lhsT=krope[:, kv, P * j:P * (j + 1)],
                                    rhs=qrope[:, h, 512 * c + lo:512 * (c + 1)],
                                    start=True, stop=True)
                                return sps

                            def emit_probs_pv(j, sps):
                                jj = j - 4 * c
                                lo = max(jj, 0) * P
                                pt = probs_pool.tile([P, 512], bf, tag="probs")
                                nc.scalar.activation(
                                    pt[:, lo:512], sps[:, lo:512],
                                    mybir.ActivationFunctionType.Exp,
                                    bias=0.0, scale=SCALE)

                                def pv(ir):
                                    nc.tensor.matmul(
                                        ops[ir][:],
                                        lhsT=pt[:, P * ir:P * (ir + 1)],
                                        rhs=vaug[:, kv, j, :],
                                        start=(j == 0),
                                        stop=(j == 4 * c + ir))

                                # PVs on fully-valid columns first (no mask
                                # dependency), then the masked diagonal PV;
                                # accumulator jj stops here, so drain it right
                                # away to free its PSUM slot early
                                for ir in range(4):
                                    if 4 * c + ir >= j and ir != jj:
                                        pv(ir)
                                if jj >= 0:
                                    nc.vector.tensor_mul(
                                        pt[:, lo:lo + P], pt[:, lo:lo + P],
                                        mask_sb[:])
                                    pv(jj)
                                    drain(jj)

                            nj = 4 * c + 4
                            depth = 3 if nj > 3 else (2 if nj > 2 else 1)
                            pending = []
                            for j in range(nj):
                                pending.append((j, emit_scores(j)))
                                if len(pending) > depth:
                                    emit_probs_pv(*pending.pop(0))
                            for pj, psps in pending:
                                emit_probs_pv(pj, psps)
                            if h + 1 < NQ:
                                emit_qproj(h + 1, head_c_order(h + 1)[ci])


_NC_CACHE = {}


def _get_nc(with_bias: bool, repeat: int = 1):
    key = (with_bias, repeat)
    if key not in _NC_CACHE:
        _NC_CACHE[key] = _build_nc(with_bias, repeat)
    return _NC_CACHE[key]


def _make_in_maps(hidden_state, wq, bq, wk, bk, wv, bv, with_bias):
    in_maps = []
    for core in range(8):
        b, hg = core // 4, core % 4
        hst = np.ascontiguousarray(
            hidden_state[b].T.astype(BF16).reshape(KT, P, S).transpose(1, 0, 2))
        wq_cols = np.concatenate(
            [wq[:, (4 * hg + h) * HD + _ORIG] for h in range(NQ)], axis=1)
        wk_cols = np.concatenate(
            [wk[:, (2 * hg + m) * HD + _ORIG] for m in range(NKV)], axis=1)
        wv_cols = wv[:, NKV * HD * hg: NKV * HD * (hg + 1)]

        def wlay(w):
            return np.ascontiguousarray(
                w.astype(BF16).reshape(KT, P, w.shape[1]).transpose(1, 0, 2))

        m = {
            "hst": hst,
            "wq": wlay(wq_cols),
            "wk": wlay(wk_cols),
            "wv": wlay(wv_cols),
            "cc": _CC,
            "ss": _SS,
            "mask": _MASK,
        }
        if with_bias:
            m["bq"] = np.ascontiguousarray(
                np.stack([bq[(4 * hg + h) * HD + _ORIG] for h in range(NQ)],
                         axis=1).astype(F32))
            m["bk"] = np.ascontiguousarray(
                np.stack([bk[(2 * hg + mm) * HD + _ORIG] for mm in range(NKV)],
                         axis=1).astype(F32))
            m["bv"] = np.ascontiguousarray(
                np.broadcast_to(
                    bv[NKV * HD * hg: NKV * HD * (hg + 1)].reshape(1, NKV, HD),
                    (P, NKV, HD)).astype(F32))
        in_maps.append(m)
    return in_maps


def _run(hidden_state, wq, bq, wk, bk, wv, bv, trace=False, **spmd_kwargs):
    hidden_state = np.asarray(hidden_state, dtype=F32)
    wq = np.asarray(wq, dtype=F32)
    wk = np.asarray(wk, dtype=F32)
    wv = np.asarray(wv, dtype=F32)
    bq = np.asarray(bq, dtype=F32)
    bk = np.asarray(bk, dtype=F32)
    bv = np.asarray(bv, dtype=F32)
    with_bias = bool(np.any(bq) or np.any(bk) or np.any(bv))
    nc = _get_nc(with_bias)
    in_maps = _make_in_maps(hidden_state, wq, bq, wk, bk, wv, bv, with_bias)
    res = run_bass_kernel_spmd(nc, in_maps, core_ids=list(range(8)),
                               trace=trace, **spmd_kwargs)
    out = np.zeros((BS, S, N_HEADS, HD), F32)
    for core in range(8):
        b, hg = core // 4, core % 4
        oc = np.asarray(res.results[core]["out"], dtype=F32)
        out[b, :, 4 * hg:4 * hg + 4, :] = oc.reshape(S, NQ, HD)
    return out, res


def kernel(hidden_state, wq, bq, wk, bk, wv, bv):
    out, _ = _run(hidden_state, wq, bq, wk, bk, wv, bv, trace=False)
    return out
